# revision 1
# baseline (speedup 1.0000x reference)
"""TRN2 Bass kernel for nn_BasePointPWL_11184094839093 (histogram_binning).

Per-channel piecewise-linear interpolation y[n,c] = PWL_c(x[n,c]) with
xp = linspace(-1,1,64) per channel (uniform breakpoints) and a learned
yp table. The reference computes

    rank = searchsorted(xp[c], x, 'left'); i = clip(rank, 1, 63)
    y = yp[c,i-1] + (x-xp[c,i-1])*(yp[c,i]-yp[c,i-1])/(xp[c,i]-xp[c,i-1]+1e-7)

In t-space t = (x+1)*31.5 the breakpoints sit at the integers 1..62, so the
whole function is an exact relu expansion with channel-independent kink
positions:

    y(t) = A_c + B_c*t + sum_{j=1..62} g_{c,j} * relu(t - j)

with coefficients computed host-side (tiny [64,63] work) from xp/yp,
folding the reference's 1e-7-regularized division exactly.  Linear
extrapolation outside [xp[0], xp[63]] matches the reference's index
clipping by construction.

Device strategy (data-parallel over 8 NeuronCores, N-axis sharding):
  - per core, x is viewed as [32, 128, 2048] natural tiles; each 128x128
    block is PE-transposed so partitions become (row-parity, channel) and
    the per-channel coefficients become per-partition scalars.
  - the ACT engine evacuates PSUM and applies t = 31.5*x + 31.5 in the
    same pass.
  - the 62-term relu sum runs on the Vector engine as custom DVE ops:
    21 paired ops (kinks (a,2a): out = acc + s0*relu(t-a) + s1*relu(t-2a),
    8 ALU stages) + 20 single-kink ops + 1 affine init, streaming at
    ~1 elem/lane/cycle; result transposed back and DMA'd out.
"""

import numpy as np

import concourse.bacc as bacc
import concourse.mybir as mybir
import concourse.tile as tile
from concourse import bass_utils
from concourse.masks import make_identity

F32 = mybir.dt.float32

N_TOTAL, C, K = 1048576, 64, 64
NCORES = 8
R = N_TOTAL // NCORES
P = 128
NBLK = 16                      # 128-blocks per compute tile (FD = 2048)

_REGISTERED = {}


def _register_custom_ops():
    if _REGISTERED:
        return _REGISTERED
    from concourse import dve_ops
    from concourse.dve_spec import Spec, Src0, Src1, C0, C1, C2, relu, lower
    from concourse.dve_uop import DveOpSpec

    def _make(name, body, reference):
        if name in dve_ops._SUB_OPCODE_FOR_NAME:
            for op in dve_ops.OPS:
                if op.name == name:
                    return op
        spec = Spec(body=body, reference=reference)
        shas = {}
        for ver in ("v3", "v4"):
            try:
                u = lower(spec, ver=ver)
                shas[ver] = DveOpSpec(name=name, opcode=0, uops=u, rd1_en=True).sha(ver)
            except Exception:
                pass
        op = dve_ops.DveOp(name, spec, subdim=False, uops_sha=shas)
        dve_ops.OPS.append(op)
        dve_ops.CUSTOM_DVE_SPECS[name] = spec
        dve_ops._SUB_OPCODE_FOR_NAME[name] = (
            dve_ops._CUSTOM_DVE_ROW_BASE + len(dve_ops.OPS) - 1
        )
        assert dve_ops._SUB_OPCODE_FOR_NAME[name] < 0x20
        return op

    # out = in1 + s0*relu(in0 - imm2) + s1*relu(in0 - 2*imm2)
    # (relu(t-2a) == relu(relu(t-a) - a) for a >= 0; reusing the imm keeps
    # the expression within the DVE's 6 carry lanes)
    _r0 = relu(Src0 - C2)
    RELU2A_FMA = _make(
        "PWL_RELU2A_FMA",
        Src1 + C0 * _r0 + C1 * relu(_r0 - C2),
        lambda in0, in1, s0, s1, imm2: in1
        + s0 * np.maximum(in0 - imm2, np.float32(0))
        + s1 * np.maximum(in0 - 2 * imm2, np.float32(0)),
    )
    # out = in1 + s0*relu(in0 - imm2)
    RELU1_FMA = _make(
        "PWL_RELU1_FMA",
        Src1 + C0 * relu(Src0 - C2),
        lambda in0, in1, s0, s1, imm2: in1
        + s0 * np.maximum(in0 - imm2, np.float32(0)),
    )
    _REGISTERED.update(RELU2A_FMA=RELU2A_FMA, RELU1_FMA=RELU1_FMA)
    return _REGISTERED


# (a, 2a) pair matching of kinks {1..62}: 21 paired ops + 20 singles.
PAIRS_2A = [
    (1, 2), (4, 8), (16, 32), (3, 6), (12, 24), (5, 10), (20, 40), (7, 14),
    (28, 56), (9, 18), (11, 22), (13, 26), (15, 30), (17, 34), (19, 38),
    (21, 42), (23, 46), (25, 50), (27, 54), (29, 58), (31, 62),
]
SINGLES_2A = [48, 36, 44, 52, 60, 33, 35, 37, 39, 41, 43, 45, 47, 49, 51, 53,
              55, 57, 59, 61]


def _host_coefficients(xp, yp):
    """[128, 64] f32: col 0 = A (alpha), col 1 = B (d0), col j+1 = g_j;
    rows tiled twice over the 64 channels."""
    xp0 = xp[0].astype(np.float64)
    Delta = 2.0 / 63.0
    dx = xp0[1:] - xp0[:-1]
    slope_x = (yp[:, 1:].astype(np.float64) - yp[:, :-1].astype(np.float64)) / (dx[None, :] + 1e-7)
    d = slope_x * Delta                               # [C, 63] t-space slopes
    coef = np.zeros((C, K), np.float64)
    coef[:, 0] = yp[:, 0]
    coef[:, 1] = d[:, 0]
    coef[:, 2:] = d[:, 1:] - d[:, :-1]                # g_j
    return np.tile(coef.astype(np.float32), (2, 1))   # [128, 64]


def _build_nc():
    ops = _register_custom_ops()
    nc = bacc.Bacc("TRN2", target_bir_lowering=False, debug=False, num_devices=NCORES)

    x_d = nc.dram_tensor("x_d", [R, C], F32, kind="ExternalInput").ap()
    coef_d = nc.dram_tensor("coef_d", [P, K], F32, kind="ExternalInput").ap()
    y_d = nc.dram_tensor("y_d", [R, C], F32, kind="ExternalOutput").ap()

    # [ntiles, 128, 128] natural tiles: partition = row-pair, free = (parity, c)
    xv = x_d.rearrange("(n a b) c -> n a (b c)", a=P, b=2)
    yv = y_d.rearrange("(n a b) c -> n a (b c)", a=P, b=2)
    ntiles = xv.shape[0]
    nouter = ntiles // NBLK
    FD = P * NBLK

    with tile.TileContext(nc) as tc:
        with (
            tc.tile_pool(name="consts", bufs=1) as consts,
            tc.tile_pool(name="io", bufs=3) as io,
            tc.tile_pool(name="work", bufs=3) as work,
            tc.tile_pool(name="ps", bufs=3, space="PSUM") as ps,
        ):
            ident = consts.tile([P, P], F32, tag="ident")
            make_identity(nc, ident)
            coef_sb = consts.tile([P, K], F32, tag="coef")
            nc.sync.dma_start(coef_sb[:], coef_d[:])

            for m in range(nouter):
                nt = io.tile([P, FD], F32, tag="nt")
                for b in range(NBLK):
                    nc.sync.dma_start(nt[:, b * P:(b + 1) * P], xv[m * NBLK + b, :, :])
                tt = work.tile([P, FD], F32, tag="tt")
                for b in range(NBLK):
                    pin = ps.tile([P, P], F32, tag="pin")
                    nc.tensor.transpose(pin[:], nt[:, b * P:(b + 1) * P], ident[:])
                    # evacuate PSUM + t = 31.5*x + 31.5 in one ACT pass
                    nc.scalar.activation(
                        tt[:, b * P:(b + 1) * P], pin[:],
                        mybir.ActivationFunctionType.Copy, bias=31.5, scale=31.5,
                    )
                acc = work.tile([P, FD], F32, tag="acc")
                nc.vector.tensor_scalar(
                    acc[:], tt[:], coef_sb[:, 1:2], coef_sb[:, 0:1],
                    mybir.AluOpType.mult, mybir.AluOpType.add,
                )
                for a, b2 in PAIRS_2A:
                    nc.vector._custom_dve(
                        ops["RELU2A_FMA"], out=acc[:], in0=tt[:], in1=acc[:],
                        s0=coef_sb[:, a + 1:a + 2], s1=coef_sb[:, b2 + 1:b2 + 2],
                        imm2=float(a),
                    )
                for a in SINGLES_2A:
                    nc.vector._custom_dve(
                        ops["RELU1_FMA"], out=acc[:], in0=tt[:], in1=acc[:],
                        s0=coef_sb[:, a + 1:a + 2], imm2=float(a),
                    )
                ot = io.tile([P, FD], F32, tag="ot")
                for b in range(NBLK):
                    pot = ps.tile([P, P], F32, tag="pot")
                    nc.tensor.transpose(pot[:], acc[:, b * P:(b + 1) * P], ident[:])
                    nc.scalar.activation(
                        ot[:, b * P:(b + 1) * P], pot[:],
                        mybir.ActivationFunctionType.Copy,
                    )
                for b in range(NBLK):
                    nc.sync.dma_start(yv[m * NBLK + b, :, :], ot[:, b * P:(b + 1) * P])

    nc.compile()
    return nc


_NC = None


def kernel(x, xp, yp):
    global _NC
    x = np.asarray(x, dtype=np.float32)
    xp = np.asarray(xp, dtype=np.float32)
    yp = np.asarray(yp, dtype=np.float32)
    assert x.shape == (N_TOTAL, C) and xp.shape == (C, K) and yp.shape == (C, K)
    coef = _host_coefficients(xp, yp)
    if _NC is None:
        _NC = _build_nc()
    in_maps = [
        {"x_d": np.ascontiguousarray(x[g * R:(g + 1) * R]), "coef_d": coef}
        for g in range(NCORES)
    ]
    res = bass_utils.run_bass_kernel_spmd(_NC, in_maps, core_ids=list(range(NCORES)))
    return np.concatenate([res.results[g]["y_d"] for g in range(NCORES)], axis=0)


# revision 3
# speedup vs baseline: 1.3338x; 1.3338x over previous
"""TRN2 Bass kernel for nn_BasePointPWL_11184094839093 (histogram_binning).

Per-channel piecewise-linear interpolation y[n,c] = PWL_c(x[n,c]) with
xp = linspace(-1,1,64) per channel (uniform breakpoints) and a learned
yp table. The reference computes

    rank = searchsorted(xp[c], x, 'left'); i = clip(rank, 1, 63)
    y = yp[c,i-1] + (x-xp[c,i-1])*(yp[c,i]-yp[c,i-1])/(xp[c,i]-xp[c,i-1]+1e-7)

In t-space t = (x+1)*31.5 the breakpoints sit at the integers 1..62, so the
whole function is an exact relu expansion with channel-independent kink
positions:

    y(t) = A_c + B_c*t + sum_{j=1..62} g_{c,j} * relu(t - j)

with coefficients computed host-side (tiny [64,63] work) from xp/yp,
folding the reference's 1e-7-regularized division exactly.  Linear
extrapolation outside [xp[0], xp[63]] matches the reference's index
clipping by construction.

Device strategy (data-parallel over 8 NeuronCores, N-axis sharding):
  - per core, x is viewed as [16, 128, 4096] natural tiles; each 128x128
    block is PE-transposed so partitions become (row-parity, channel) and
    the per-channel coefficients become per-partition scalars.
  - the ACT engine evacuates PSUM and applies t = 31.5*x + 31.5 in the
    same pass, and produces shifted copies t-S for the paired kinks.
  - the 62-term relu sum runs on the Vector engine as 31 paired custom DVE
    ops (out = acc + s0*relu(t'-a) + s1*relu(t'-2a), 8 ALU stages, via
    relu(u-2a) == relu(relu(u-a)-a)) + 1 affine init, streaming at
    ~1 elem/lane/cycle at ~98% occupancy; result transposed back and
    DMA'd out.  This sits at the DVE scalar-port floor: each op can carry
    at most two per-partition coefficients, so 62 kinks need >= 31 ops.
"""

import numpy as np

import concourse.bacc as bacc
import concourse.mybir as mybir
import concourse.tile as tile
from concourse import bass_utils
from concourse.masks import make_identity

F32 = mybir.dt.float32

N_TOTAL, C, K = 1048576, 64, 64
NCORES = 8
R = N_TOTAL // NCORES
P = 128
NBLK = 32                      # 128-blocks per compute tile (FD = 4096)

_REGISTERED = {}


def _register_custom_ops():
    if _REGISTERED:
        return _REGISTERED
    from concourse import dve_ops
    from concourse.dve_spec import Spec, Src0, Src1, C0, C1, C2, relu, lower
    from concourse.dve_uop import DveOpSpec

    def _make(name, body, reference):
        if name in dve_ops._SUB_OPCODE_FOR_NAME:
            for op in dve_ops.OPS:
                if op.name == name:
                    return op
        spec = Spec(body=body, reference=reference)
        shas = {}
        for ver in ("v3", "v4"):
            try:
                u = lower(spec, ver=ver)
                shas[ver] = DveOpSpec(name=name, opcode=0, uops=u, rd1_en=True).sha(ver)
            except Exception:
                pass
        op = dve_ops.DveOp(name, spec, subdim=False, uops_sha=shas)
        dve_ops.OPS.append(op)
        dve_ops.CUSTOM_DVE_SPECS[name] = spec
        dve_ops._SUB_OPCODE_FOR_NAME[name] = (
            dve_ops._CUSTOM_DVE_ROW_BASE + len(dve_ops.OPS) - 1
        )
        assert dve_ops._SUB_OPCODE_FOR_NAME[name] < 0x20
        return op

    # out = in1 + s0*relu(in0 - imm2) + s1*relu(in0 - 2*imm2)
    # (relu(t-2a) == relu(relu(t-a) - a) for a >= 0; reusing the imm keeps
    # the expression within the DVE's 6 carry lanes)
    _r0 = relu(Src0 - C2)
    RELU2A_FMA = _make(
        "PWL_RELU2A_FMA",
        Src1 + C0 * _r0 + C1 * relu(_r0 - C2),
        lambda in0, in1, s0, s1, imm2: in1
        + s0 * np.maximum(in0 - imm2, np.float32(0))
        + s1 * np.maximum(in0 - 2 * imm2, np.float32(0)),
    )
    # out = in1 + s0*relu(in0 - imm2)
    RELU1_FMA = _make(
        "PWL_RELU1_FMA",
        Src1 + C0 * relu(Src0 - C2),
        lambda in0, in1, s0, s1, imm2: in1
        + s0 * np.maximum(in0 - imm2, np.float32(0)),
    )
    _REGISTERED.update(RELU2A_FMA=RELU2A_FMA, RELU1_FMA=RELU1_FMA)
    return _REGISTERED


# (a, 2a) pair matching of kinks {1..62}: 21 paired ops + 20 singles.
PAIRS_2A = [
    (1, 2), (4, 8), (16, 32), (3, 6), (12, 24), (5, 10), (20, 40), (7, 14),
    (28, 56), (9, 18), (11, 22), (13, 26), (15, 30), (17, 34), (19, 38),
    (21, 42), (23, 46), (25, 50), (27, 54), (29, 58), (31, 62),
]
SINGLES_2A = [48, 36, 44, 52, 60, 33, 35, 37, 39, 41, 43, 45, 47, 49, 51, 53,
              55, 57, 59, 61]

# Full 31-pair matching: kink pairs (p, q) with q = 2p - S are evaluated on a
# shifted copy t' = t - S (produced by the otherwise-idle ACT engine), where
# the (a, 2a) relu nesting applies with a' = p - S > 0.
SHIFT_PAIRS = [
    (0, PAIRS_2A),
    (23, [(33, 43), (35, 47), (37, 51), (39, 55), (41, 59), (36, 49)]),
    (36, [(44, 52), (48, 60)]),
    (37, [(45, 53)]),
    (53, [(57, 61)]),
]
_cov = sorted([k for _, ps in SHIFT_PAIRS for pq in ps for k in pq])
assert _cov == list(range(1, 63)), _cov


def _host_coefficients(xp, yp):
    """[128, 64] f32: col 0 = A (alpha), col 1 = B (d0), col j+1 = g_j;
    rows tiled twice over the 64 channels."""
    xp0 = xp[0].astype(np.float64)
    Delta = 2.0 / 63.0
    dx = xp0[1:] - xp0[:-1]
    slope_x = (yp[:, 1:].astype(np.float64) - yp[:, :-1].astype(np.float64)) / (dx[None, :] + 1e-7)
    d = slope_x * Delta                               # [C, 63] t-space slopes
    coef = np.zeros((C, K), np.float64)
    coef[:, 0] = yp[:, 0]
    coef[:, 1] = d[:, 0]
    coef[:, 2:] = d[:, 1:] - d[:, :-1]                # g_j
    return np.tile(coef.astype(np.float32), (2, 1))   # [128, 64]


def _build_nc():
    ops = _register_custom_ops()
    nc = bacc.Bacc("TRN2", target_bir_lowering=False, debug=False, num_devices=NCORES)

    x_d = nc.dram_tensor("x_d", [R, C], F32, kind="ExternalInput").ap()
    coef_d = nc.dram_tensor("coef_d", [P, K], F32, kind="ExternalInput").ap()
    y_d = nc.dram_tensor("y_d", [R, C], F32, kind="ExternalOutput").ap()

    # [ntiles, 128, 128] natural tiles: partition = row-pair, free = (parity, c)
    xv = x_d.rearrange("(n a b) c -> n a (b c)", a=P, b=2)
    yv = y_d.rearrange("(n a b) c -> n a (b c)", a=P, b=2)
    ntiles = xv.shape[0]
    nouter = ntiles // NBLK
    FD = P * NBLK

    with tile.TileContext(nc) as tc:
        with (
            tc.tile_pool(name="consts", bufs=1) as consts,
            tc.tile_pool(name="io", bufs=2) as io,
            tc.tile_pool(name="work", bufs=2) as work,
            tc.tile_pool(name="shf", bufs=2) as shf,
            tc.tile_pool(name="ps", bufs=3, space="PSUM") as ps,
        ):
            ident = consts.tile([P, P], F32, tag="ident")
            make_identity(nc, ident)
            coef_sb = consts.tile([P, K], F32, tag="coef")
            nc.sync.dma_start(coef_sb[:], coef_d[:])

            for m in range(nouter):
                nt = io.tile([P, FD], F32, tag="nt")
                for b in range(NBLK):
                    nc.sync.dma_start(nt[:, b * P:(b + 1) * P], xv[m * NBLK + b, :, :])
                tt = work.tile([P, FD], F32, tag="tt")
                for b in range(NBLK):
                    pin = ps.tile([P, P], F32, tag="pin")
                    nc.tensor.transpose(pin[:], nt[:, b * P:(b + 1) * P], ident[:])
                    # evacuate PSUM + t = 31.5*x + 31.5 in one ACT pass
                    nc.scalar.activation(
                        tt[:, b * P:(b + 1) * P], pin[:],
                        mybir.ActivationFunctionType.Copy, bias=31.5, scale=31.5,
                    )
                acc = work.tile([P, FD], F32, tag="acc")
                nc.vector.tensor_scalar(
                    acc[:], tt[:], coef_sb[:, 1:2], coef_sb[:, 0:1],
                    mybir.AluOpType.mult, mybir.AluOpType.add,
                )
                for S, pairs in SHIFT_PAIRS:
                    if S == 0:
                        src_t = tt
                    else:
                        src_t = shf.tile([P, FD], F32, tag="shf")
                        nc.scalar.activation(
                            src_t[:], tt[:], mybir.ActivationFunctionType.Copy,
                            bias=-float(S), scale=1.0,
                        )
                    for p, q in pairs:
                        nc.vector._custom_dve(
                            ops["RELU2A_FMA"], out=acc[:], in0=src_t[:], in1=acc[:],
                            s0=coef_sb[:, p + 1:p + 2], s1=coef_sb[:, q + 1:q + 2],
                            imm2=float(p - S),
                        )
                ot = io.tile([P, FD], F32, tag="ot")
                for b in range(NBLK):
                    pot = ps.tile([P, P], F32, tag="pot")
                    nc.tensor.transpose(pot[:], acc[:, b * P:(b + 1) * P], ident[:])
                    nc.scalar.activation(
                        ot[:, b * P:(b + 1) * P], pot[:],
                        mybir.ActivationFunctionType.Copy,
                    )
                for b in range(NBLK):
                    nc.sync.dma_start(yv[m * NBLK + b, :, :], ot[:, b * P:(b + 1) * P])

    nc.compile()
    return nc


_NC = None


def kernel(x, xp, yp):
    global _NC
    x = np.asarray(x, dtype=np.float32)
    xp = np.asarray(xp, dtype=np.float32)
    yp = np.asarray(yp, dtype=np.float32)
    assert x.shape == (N_TOTAL, C) and xp.shape == (C, K) and yp.shape == (C, K)
    coef = _host_coefficients(xp, yp)
    if _NC is None:
        _NC = _build_nc()
    in_maps = [
        {"x_d": np.ascontiguousarray(x[g * R:(g + 1) * R]), "coef_d": coef}
        for g in range(NCORES)
    ]
    res = bass_utils.run_bass_kernel_spmd(_NC, in_maps, core_ids=list(range(NCORES)))
    return np.concatenate([res.results[g]["y_d"] for g in range(NCORES)], axis=0)


# revision 4
# speedup vs baseline: 1.3510x; 1.0129x over previous
"""TRN2 Bass kernel for nn_BasePointPWL_11184094839093 (histogram_binning).

Per-channel piecewise-linear interpolation y[n,c] = PWL_c(x[n,c]) with
xp = linspace(-1,1,64) per channel (uniform breakpoints) and a learned
yp table. The reference computes

    rank = searchsorted(xp[c], x, 'left'); i = clip(rank, 1, 63)
    y = yp[c,i-1] + (x-xp[c,i-1])*(yp[c,i]-yp[c,i-1])/(xp[c,i]-xp[c,i-1]+1e-7)

In t-space t = (x+1)*31.5 the breakpoints sit at the integers 1..62, so the
whole function is an exact relu expansion with channel-independent kink
positions:

    y(t) = A_c + B_c*t + sum_{j=1..62} g_{c,j} * relu(t - j)

with coefficients computed host-side (tiny [64,63] work) from xp/yp,
folding the reference's 1e-7-regularized division exactly.  Linear
extrapolation outside [xp[0], xp[63]] matches the reference's index
clipping by construction.

Device strategy (data-parallel over 8 NeuronCores, N-axis sharding):
  - per core, x is viewed as [16, 128, 4096] natural tiles; each 128x128
    block is PE-transposed so partitions become (row-parity, channel) and
    the per-channel coefficients become per-partition scalars.
  - the ACT engine evacuates PSUM twice per block: once applying
    t = 31.5*x + 31.5, once initializing the accumulator with the fused
    affine acc0 = B*t + A (per-partition scale/bias); it also produces
    the shifted copies t-S for the paired kinks.
  - the 62-term relu sum runs on the Vector engine as exactly 31 paired
    custom DVE ops (out = acc + s0*relu(t'-a) + s1*relu(t'-2a), 8 ALU
    stages, via relu(u-2a) == relu(relu(u-a)-a)), streaming at
    ~1 elem/lane/cycle at ~98% occupancy; result transposed back and
    DMA'd out.  This sits at the DVE scalar-port floor: each op can carry
    at most two per-partition coefficients, so 62 kinks need >= 31 ops.
"""

import numpy as np

import concourse.bacc as bacc
import concourse.mybir as mybir
import concourse.tile as tile
from concourse import bass_utils
from concourse.masks import make_identity

F32 = mybir.dt.float32

N_TOTAL, C, K = 1048576, 64, 64
NCORES = 8
R = N_TOTAL // NCORES
P = 128
NBLK = 32                      # 128-blocks per compute tile (FD = 4096)

_REGISTERED = {}


def _register_custom_ops():
    if _REGISTERED:
        return _REGISTERED
    from concourse import dve_ops
    from concourse.dve_spec import Spec, Src0, Src1, C0, C1, C2, relu, lower
    from concourse.dve_uop import DveOpSpec

    def _make(name, body, reference):
        if name in dve_ops._SUB_OPCODE_FOR_NAME:
            for op in dve_ops.OPS:
                if op.name == name:
                    return op
        spec = Spec(body=body, reference=reference)
        shas = {}
        for ver in ("v3", "v4"):
            try:
                u = lower(spec, ver=ver)
                shas[ver] = DveOpSpec(name=name, opcode=0, uops=u, rd1_en=True).sha(ver)
            except Exception:
                pass
        op = dve_ops.DveOp(name, spec, subdim=False, uops_sha=shas)
        dve_ops.OPS.append(op)
        dve_ops.CUSTOM_DVE_SPECS[name] = spec
        dve_ops._SUB_OPCODE_FOR_NAME[name] = (
            dve_ops._CUSTOM_DVE_ROW_BASE + len(dve_ops.OPS) - 1
        )
        assert dve_ops._SUB_OPCODE_FOR_NAME[name] < 0x20
        return op

    # out = in1 + s0*relu(in0 - imm2) + s1*relu(in0 - 2*imm2)
    # (relu(t-2a) == relu(relu(t-a) - a) for a >= 0; reusing the imm keeps
    # the expression within the DVE's 6 carry lanes)
    _r0 = relu(Src0 - C2)
    RELU2A_FMA = _make(
        "PWL_RELU2A_FMA",
        Src1 + C0 * _r0 + C1 * relu(_r0 - C2),
        lambda in0, in1, s0, s1, imm2: in1
        + s0 * np.maximum(in0 - imm2, np.float32(0))
        + s1 * np.maximum(in0 - 2 * imm2, np.float32(0)),
    )
    # out = in1 + s0*relu(in0 - imm2)
    RELU1_FMA = _make(
        "PWL_RELU1_FMA",
        Src1 + C0 * relu(Src0 - C2),
        lambda in0, in1, s0, s1, imm2: in1
        + s0 * np.maximum(in0 - imm2, np.float32(0)),
    )
    _REGISTERED.update(RELU2A_FMA=RELU2A_FMA, RELU1_FMA=RELU1_FMA)
    return _REGISTERED


# (a, 2a) pair matching of kinks {1..62}: 21 paired ops + 20 singles.
PAIRS_2A = [
    (1, 2), (4, 8), (16, 32), (3, 6), (12, 24), (5, 10), (20, 40), (7, 14),
    (28, 56), (9, 18), (11, 22), (13, 26), (15, 30), (17, 34), (19, 38),
    (21, 42), (23, 46), (25, 50), (27, 54), (29, 58), (31, 62),
]
SINGLES_2A = [48, 36, 44, 52, 60, 33, 35, 37, 39, 41, 43, 45, 47, 49, 51, 53,
              55, 57, 59, 61]

# Full 31-pair matching: kink pairs (p, q) with q = 2p - S are evaluated on a
# shifted copy t' = t - S (produced by the otherwise-idle ACT engine), where
# the (a, 2a) relu nesting applies with a' = p - S > 0.
SHIFT_PAIRS = [
    (0, PAIRS_2A),
    (23, [(33, 43), (35, 47), (37, 51), (39, 55), (41, 59), (36, 49)]),
    (36, [(44, 52), (48, 60)]),
    (37, [(45, 53)]),
    (53, [(57, 61)]),
]
_cov = sorted([k for _, ps in SHIFT_PAIRS for pq in ps for k in pq])
assert _cov == list(range(1, 63)), _cov


def _host_coefficients(xp, yp):
    """[128, 64] f32: col 0 = A (alpha), col 1 = B (d0), col j+1 = g_j;
    rows tiled twice over the 64 channels."""
    xp0 = xp[0].astype(np.float64)
    Delta = 2.0 / 63.0
    dx = xp0[1:] - xp0[:-1]
    slope_x = (yp[:, 1:].astype(np.float64) - yp[:, :-1].astype(np.float64)) / (dx[None, :] + 1e-7)
    d = slope_x * Delta                               # [C, 63] t-space slopes
    coef = np.zeros((C, K), np.float64)
    A = yp[:, 0].astype(np.float64)
    B = d[:, 0]
    coef[:, 0] = 31.5 * B                             # fused init scale (on x)
    coef[:, 1] = 31.5 * B + A                         # fused init bias
    coef[:, 2:] = d[:, 1:] - d[:, :-1]                # g_j
    return np.tile(coef.astype(np.float32), (2, 1))   # [128, 64]


def _build_nc():
    ops = _register_custom_ops()
    nc = bacc.Bacc("TRN2", target_bir_lowering=False, debug=False, num_devices=NCORES)

    x_d = nc.dram_tensor("x_d", [R, C], F32, kind="ExternalInput").ap()
    coef_d = nc.dram_tensor("coef_d", [P, K], F32, kind="ExternalInput").ap()
    y_d = nc.dram_tensor("y_d", [R, C], F32, kind="ExternalOutput").ap()

    # [ntiles, 128, 128] natural tiles: partition = row-pair, free = (parity, c)
    xv = x_d.rearrange("(n a b) c -> n a (b c)", a=P, b=2)
    yv = y_d.rearrange("(n a b) c -> n a (b c)", a=P, b=2)
    ntiles = xv.shape[0]
    nouter = ntiles // NBLK
    FD = P * NBLK

    with tile.TileContext(nc) as tc:
        with (
            tc.tile_pool(name="consts", bufs=1) as consts,
            tc.tile_pool(name="io", bufs=2) as io,
            tc.tile_pool(name="work", bufs=2) as work,
            tc.tile_pool(name="shf", bufs=2) as shf,
            tc.tile_pool(name="ps", bufs=3, space="PSUM") as ps,
        ):
            ident = consts.tile([P, P], F32, tag="ident")
            make_identity(nc, ident)
            coef_sb = consts.tile([P, K], F32, tag="coef")
            nc.sync.dma_start(coef_sb[:], coef_d[:])

            for m in range(nouter):
                nt = io.tile([P, FD], F32, tag="nt")
                for b in range(NBLK):
                    nc.sync.dma_start(nt[:, b * P:(b + 1) * P], xv[m * NBLK + b, :, :])
                tt = work.tile([P, FD], F32, tag="tt")
                acc = work.tile([P, FD], F32, tag="acc")
                for b in range(NBLK):
                    pin = ps.tile([P, P], F32, tag="pin")
                    nc.tensor.transpose(pin[:], nt[:, b * P:(b + 1) * P], ident[:])
                    # evacuate PSUM + t = 31.5*x + 31.5 in one ACT pass
                    nc.scalar.activation(
                        tt[:, b * P:(b + 1) * P], pin[:],
                        mybir.ActivationFunctionType.Copy, bias=31.5, scale=31.5,
                    )
                    # second evac of the same PSUM initializes the accumulator:
                    # acc0 = B*t + A = (31.5*B)*x + (31.5*B + A)
                    # (coef col 0 = fused scale, col 1 = fused bias)
                    nc.scalar.activation(
                        acc[:, b * P:(b + 1) * P], pin[:],
                        mybir.ActivationFunctionType.Identity,
                        bias=coef_sb[:, 1:2], scale=coef_sb[:, 0:1],
                    )
                for S, pairs in SHIFT_PAIRS:
                    if S == 0:
                        src_t = tt
                    else:
                        src_t = shf.tile([P, FD], F32, tag="shf")
                        nc.scalar.activation(
                            src_t[:], tt[:], mybir.ActivationFunctionType.Copy,
                            bias=-float(S), scale=1.0,
                        )
                    for p, q in pairs:
                        nc.vector._custom_dve(
                            ops["RELU2A_FMA"], out=acc[:], in0=src_t[:], in1=acc[:],
                            s0=coef_sb[:, p + 1:p + 2], s1=coef_sb[:, q + 1:q + 2],
                            imm2=float(p - S),
                        )
                ot = io.tile([P, FD], F32, tag="ot")
                for b in range(NBLK):
                    pot = ps.tile([P, P], F32, tag="pot")
                    nc.tensor.transpose(pot[:], acc[:, b * P:(b + 1) * P], ident[:])
                    nc.scalar.activation(
                        ot[:, b * P:(b + 1) * P], pot[:],
                        mybir.ActivationFunctionType.Copy,
                    )
                for b in range(NBLK):
                    nc.sync.dma_start(yv[m * NBLK + b, :, :], ot[:, b * P:(b + 1) * P])

    nc.compile()
    return nc


_NC = None


def kernel(x, xp, yp):
    global _NC
    x = np.asarray(x, dtype=np.float32)
    xp = np.asarray(xp, dtype=np.float32)
    yp = np.asarray(yp, dtype=np.float32)
    assert x.shape == (N_TOTAL, C) and xp.shape == (C, K) and yp.shape == (C, K)
    coef = _host_coefficients(xp, yp)
    if _NC is None:
        _NC = _build_nc()
    in_maps = [
        {"x_d": np.ascontiguousarray(x[g * R:(g + 1) * R]), "coef_d": coef}
        for g in range(NCORES)
    ]
    res = bass_utils.run_bass_kernel_spmd(_NC, in_maps, core_ids=list(range(NCORES)))
    return np.concatenate([res.results[g]["y_d"] for g in range(NCORES)], axis=0)
